# revision 27
# baseline (speedup 1.0000x reference)
"""Self-contained Trainium2 Bass kernel for nn_GCNResnet (batch-attention GCN).

Math (reference collapse):
  out[b,:] = sum_n c_n * softmax(X_n X_n^T)[b,:] @ (X_n @ W) + offset*(1_D @ W)
where X_n = x[:, n, :] ([B=4096, D=10]); c_n and offset fold BN(eval) +
adjacency + GCN + avgpool scalars. Per node the softmax normalizer is folded
into the PV matmul via a ones column:
  U_n = E_n @ [c_n*(X_n@W) | 1]   ->   out_n = U[:, :10] / U[:, 10]
(no max-subtraction: |scores| <= ~45 << 88, exp stays in bf16-exponent range).

Sharding: row-slab parallel over 8 cores (512 query rows each, full keys),
no collectives.

Design (PE pinned at ~1.2GHz on this part; per-BANK PSUM write ports):
  * All PE operands bf16, host-precast -> all loads are plain flat-descriptor
    HWDGE/SWDGE DMAs (~100GB/s aggregate fabric; 2-level-AP replica DMAs
    collapse to ~40GB/s and are avoided).
  * Scores are 4-way PE row-tiled: strip r (array rows 32r..32r+10) handles
    key chunks ck%4==r, so the 4 chunks of a score group stream CONCURRENTLY
    into 4 different PSUM banks. Key data is strip-compacted (each strip
    loads only its quarter of the keys); only the small query block is
    replicated across the 4 partition offsets.
  * The scores matmul emits t = C1*s + C2 (C1=128/ln2, C2=127*128) via
    C1-prescaled keys plus an 11th contraction row (C2 x ones).
  * exp alternates whole PSUM-buffer groups between two engines:
      - ACT (4-bank groups): native Exp, free affine undoes C1/C2, bf16 out.
      - DVE (3-bank groups): ONE tensor_copy f32->int16 with rounding; the
        int16 bits ARE bf16(exp(s)) (Schraudolph: i16 = 128*e + m).
    One engine per group => each PV group depends on a single exp
    instruction and issues as one column-tiled batch.
  * PV (M=11) is column-tiled 4x: chunk i of a group lands in PE column
    group 32i, so 4 chunks share PSUM write-port cycles. Partials accumulate
    at psum partitions 32g+d (host sums the 4 groups, divides by the folded
    denominator row, adds the BN/adjacency bias).
  * PSUM: scores ping-pong 4+3 banks with GLOBALLY alternating parity
    (no same-buffer back-to-back across node boundaries), U accumulator
    1 bank (= 8). Emission is software-pipelined (scores g+1 before PV g).
  * Steady state is bound by the A-buffer round-trip: sc_A 0.6us + drain/sem
    0.7us + ACT exp 2.0us  ~= 3.3us per 7 key chunks.
"""

import sys

if "/opt/trn_rl_repo" not in sys.path:
    sys.path.insert(0, "/opt/trn_rl_repo")

import numpy as np
import ml_dtypes

import concourse.bass as bass
import concourse.mybir as mybir
from concourse import tile
from concourse.bass_utils import run_bass_kernel_spmd
from concourse.vector_clock import ScopedClock

B, N, D = 4096, 3, 10
NCORES = 8
R = B // NCORES            # 512 query rows per core
KC = B // 128              # 32 key chunks of 128
DP = D + 1                 # data rows + folded ones/bias row
BN_EPS = 1e-5

C1 = 128.0 / float(np.log(2.0))   # Schraudolph scale: t = C1*s + C2
C2 = 127.0 * 128.0

# perf/numerics switches (exercised by test.py)
GROUP_PATTERN = (4, 3)     # scores-group widths; psum buffers ping-pong
N_ROWREP = 4               # scores row-tiling degree (PE strips / replicas)
N_COLGRP = 4               # PV column-tiling degree (1, 2 or 4)
ET_BUFS = 4
U_MEMSET = True            # DVE-memset U bank per node (robust has_written)

# exp split: ACT columns per group width, balancing (x+352)/1.2GHz against
# ((W-x)+150)/0.96GHz (measured instruction overheads)
ACT_COLS = {2048: 1065, 1536: 780, 1024: 524, 512: 296}


def _patched_drain_and_barrier(self, tick_clock, wait_clock):
    # Walrus in this container rejects >1 sync-wait on a CTRL-class
    # instruction; absorb the tail-drain waits into SP nops, one wait each.
    nc = self.nc
    probe = nc.sync.nop()
    wait_clock.add_sem_waits(probe.ins, ScopedClock({None: tick_clock.global_clock}))
    si = probe.ins.sync_info
    waits = list(si.on_wait) if si is not None else []
    upds = list(si.on_update) if si is not None else []
    probe.ins.sync_info = mybir.SyncInfo(on_wait=waits[:1], on_update=upds)
    for w in waits[1:]:
        n = nc.sync.nop()
        n.ins.sync_info = mybir.SyncInfo(on_wait=[w], on_update=[])
    nc.sync.drain()
    nc.all_engine_barrier()
    assert self.sems is not None
    popped = nc._tile_sem_poison_stack.pop()
    assert popped is self._sem_poison
    nc.clear_and_free_semaphores(list(self.sems.allocated().values()))
    nc.all_engine_barrier()


tile.TileContext._drain_and_barrier = _patched_drain_and_barrier

_MAX_WAITS = 1
_waitsplit_ctr = [0]


def _split_sync_waits(nc):
    """Walrus here allows very few sync-waits per instruction. Move excess
    waits onto same-engine no-ops placed immediately before the instruction
    (engine streams are in-order, so semantics are preserved)."""
    for f in nc.m.functions:
        for bb in f.blocks:
            new = []
            changed = False
            for inst in bb.instructions:
                si = inst.sync_info
                waits = list(si.on_wait) if si is not None else []
                if len(waits) > _MAX_WAITS:
                    changed = True
                    for w in waits[:-_MAX_WAITS]:
                        _waitsplit_ctr[0] += 1
                        nop = mybir.InstNoOp(
                            name=f"I-waitsplit-{_waitsplit_ctr[0]}", ins=[], outs=[]
                        )
                        nop.engine = inst.engine
                        nop.sync_info = mybir.SyncInfo(on_wait=[w], on_update=[])
                        new.append(nop)
                    inst.sync_info = mybir.SyncInfo(
                        on_wait=waits[-_MAX_WAITS:], on_update=list(si.on_update)
                    )
                new.append(inst)
            if changed:
                bb.instructions = new


def _groups_all():
    """Per-node chunk groups with GLOBALLY alternating psum-buffer parity
    (width follows parity: 4 banks for parity 0, 3 for parity 1), so
    consecutive groups never reuse the same buffer - including across node
    boundaries. Truncated at node ends; occasional small trailing groups."""
    out = []
    parity = 0
    for n in range(N):
        gs, c = [], 0
        while c < KC:
            w = min(GROUP_PATTERN[parity], KC - c)
            gs.append((list(range(c, c + w)), parity))
            c += w
            parity ^= 1
        out.append(gs)
    return out


def build_nc(rep: int = 1, rep_marker: bool = False, mode: str = "full") -> bass.Bass:
    """One-core SPMD program: full keys replicated, this core's 512-row slab.

    mode: "full" (loads+compute per rep), "loads" (DMAs only per rep),
    "compute" (loads once, compute per rep), "nopv" (no PV matmuls).
    """
    f32 = mybir.dt.float32
    bf16 = mybir.dt.bfloat16
    i16 = mybir.dt.int16
    nc = bass.Bass()

    # register the exp-affine bias as a const AP (activation() requires
    # non-zero float biases to be SBUF-resident)
    bias_val = -(C2 / C1)
    _bt = nc.alloc_sbuf_tensor("const-exp-bias", [128, 1], f32)
    nc.gpsimd.memset(_bt.ap(), bias_val)
    nc.const_aps.aps[(f32, bias_val)] = _bt.ap()

    # bf16 inputs, host-precast. xk rows 0..9 = C1*x^T (all keys), row 10 = C2.
    # xq rows 0..9 = x^T (this core's slab), row 10 = 1. The PE row-tiling
    # strips need copies at partition offsets 32*r: done as N_ROWREP flat
    # DMAs from the same DRAM source (flat per-partition descriptors run at
    # ~130GB/s; 2-level-AP replica DMAs collapse to ~40GB/s). xh is
    # partition-major so its descriptors are one contiguous run per partition.
    KPS = B // N_ROWREP   # keys per strip (chunks ck % N_ROWREP == r)
    xkq = nc.declare_dram_parameter(
        "xkq", [N, N_ROWREP, DP, KPS + R], bf16, isOutput=False
    )
    xh = nc.declare_dram_parameter("xh", [128, N * KC * DP], bf16, isOutput=False)
    uout = nc.declare_dram_parameter("uout", [128, N * R + 4], f32, isOutput=True)

    groups_all = _groups_all()
    wmax = max(GROUP_PATTERN)

    with tile.TileContext(nc) as tc:
        with (
            tc.tile_pool(name="xkp", bufs=1) as xkp,
            tc.tile_pool(name="xqp", bufs=1) as xqp,
            tc.tile_pool(name="xhp", bufs=1) as xhp,
            tc.tile_pool(name="etp", bufs=ET_BUFS) as etp,
            tc.tile_pool(name="usb", bufs=2) as usbp,
            tc.tile_pool(name="mrk", bufs=1) as mrkp,
            tc.tile_pool(name="pssA", bufs=1, space="PSUM") as pssA,
            tc.tile_pool(name="pssB", bufs=1, space="PSUM") as pssB,
            tc.tile_pool(name="psu", bufs=1, space="PSUM") as psu,
        ):
            xk_sb = xq_sb = xh_sb = None
            for rep_i in range(rep):
                if mode != "compute" or rep_i == 0:
                    # ---- input loads (plain HWDGE DMAs, bf16) ----
                    # xkq: keys||queries per node; strip replica r lives at
                    # partitions 32r..32r+10. One flat [11, B+R] DMA per
                    # (node, strip), node-major order so node 0 is compute-
                    # ready first; replicas alternate sync/scalar queues
                    # (flat descriptors ~130GB/s; 2-level-AP replicas
                    # collapse to ~40GB/s). xh goes on the gpsimd queue.
                    xkq_sb = xkp.tile(
                        [128, N * (KPS + R)], bf16, tag="xkq", name="xkq"
                    )
                    xh_sb = xhp.tile([128, N * KC * DP], bf16, tag="xh", name="xh")
                    # node 0's four strips land in PARALLEL (sync/scalar/
                    # gpsimd x2) so the first score group isn't serialized
                    # behind one ~50GB/s queue; xh follows on sync (needed
                    # by the first PV ~2 groups later); nodes 1-2 trail on
                    # sync/gpsimd. scalar (ACT) gets only one early issue.
                    W1 = KPS + R

                    def ld(eng, n, r):
                        po = 32 * r
                        eng.dma_start(
                            xkq_sb[po : po + DP, n * W1 : (n + 1) * W1], xkq[n, r]
                        )

                    ld(nc.sync, 0, 0)
                    ld(nc.scalar, 0, 1)
                    ld(nc.sync, 0, 2)
                    ld(nc.scalar, 0, 3)
                    nc.sync.dma_start(xh_sb[:], xh[:])
                    for r in range(N_ROWREP):
                        ld(nc.sync, 1, r)
                    for r in range(N_ROWREP):
                        ld(nc.gpsimd, 2, r)
                if mode == "loads":
                    continue

                for n in range(N):
                    groups = groups_all[n]
                    u_ps = psu.tile([128, R], f32, tag="u")
                    if U_MEMSET:
                        nc.vector.memset(u_ps[:], 0.0)
                    else:
                        # 1-element dummy matmul with start=True: flash-clears
                        # the whole U bank's has_written bits, so the PV MMs
                        # (start=False) overwrite stale values on first touch.
                        # Its own 1.0 lands at psum[0,0] (bit set -> later
                        # accumulated); the host subtracts it in _finish.
                        one = nc.const_aps.aps[(bf16, 1.0)]
                        nc.tensor.matmul(
                            u_ps[0:1, 0:1],
                            lhsT=one[0:1, 0:1],
                            rhs=one[0:1, 0:1],
                            start=True,
                            stop=False,
                            skip_group_check=True,
                        )
                    # software pipeline: emit scores(g), then exp(g), then
                    # PV(g-1) so the PE stream has scores(g+1) queued while
                    # exp(g) runs.
                    pending_pv = None
                    for g_i, (g, parity) in enumerate(groups + [(None, None)]):
                        if g is not None:
                            w = len(g)
                            pool = pssB if parity else pssA
                            gw = GROUP_PATTERN[parity % len(GROUP_PATTERN)]
                            ps = pool.tile(
                                [128, R * gw], f32, tag=f"s{parity}",
                                name=f"s{parity}",
                            )
                            W1 = KPS + R
                            for i, ck in enumerate(g):
                                po = 32 * (ck % N_ROWREP)
                                ko = n * W1 + 128 * (ck // N_ROWREP)
                                qo = n * W1 + KPS
                                nc.tensor.matmul(
                                    ps[:, R * i : R * (i + 1)],
                                    lhsT=xkq_sb[po : po + DP, ko : ko + 128],
                                    rhs=xkq_sb[po : po + DP, qo : qo + R],
                                    tile_position=(po, 0),
                                )
                            # exp: whole-group engine assignment - ACT owns
                            # the 4-bank groups (native Exp, affine undoes
                            # C1/C2), DVE the 3-bank groups (convert f32->i16
                            # == bf16 bits of exp(s), Schraudolph). One engine
                            # per group means each PV group depends on a
                            # single exp instruction and issues as one
                            # column-tiled batch.
                            et = etp.tile([128, R * wmax], bf16, tag="et")
                            et_i16 = et[:].bitcast(i16)
                            cols = R * w
                            if parity == 0:
                                nc.scalar.activation(
                                    et[:, 0:cols],
                                    ps[:, 0:cols],
                                    mybir.ActivationFunctionType.Exp,
                                    bias=bias_val,
                                    scale=1.0 / C1,
                                )
                            else:
                                nc.vector.tensor_copy(
                                    et_i16[:, 0:cols], ps[:, 0:cols]
                                )
                            cur = (g, et)
                        else:
                            cur = None
                        if pending_pv is not None and mode != "nopv":
                            pg, pet = pending_pv
                            for i, ck in enumerate(pg):
                                cg = i % N_COLGRP
                                nc.tensor.matmul(
                                    u_ps[32 * cg : 32 * cg + DP, :],
                                    lhsT=xh_sb[
                                        :,
                                        (n * KC + ck) * DP : (n * KC + ck + 1) * DP,
                                    ],
                                    rhs=pet[:, R * i : R * (i + 1)],
                                    start=False,
                                    stop=(ck == KC - 1),
                                    tile_position=(0, 32 * cg),
                                    skip_group_check=True,
                                )
                        pending_pv = cur
                    if mode == "nopv":
                        continue
                    # drain this node's U partials [128, 512] to SBUF (split
                    # between ACT and DVE) and ship to DRAM; host combines
                    # column groups, divides, transposes.
                    u_sb = usbp.tile([128, R], f32, tag="usb", name="u_sb")
                    h = R // 2
                    # both halves on DVE: ACT's queue stays clear so the next
                    # node's first exp isn't delayed behind a copy
                    nc.vector.tensor_copy(u_sb[:, 0:h], u_ps[:, 0:h])
                    nc.vector.tensor_copy(u_sb[:, h:R], u_ps[:, h:R])
                    if n == N - 1:
                        # last node: exp is done on ACT; split the drain
                        # across two queues to shorten the kernel tail
                        nc.sync.dma_start(
                            uout[:, R * n : R * n + h], u_sb[:, 0:h]
                        )
                        nc.scalar.dma_start(
                            uout[:, R * n + h : R * (n + 1)], u_sb[:, h:R]
                        )
                    else:
                        nc.sync.dma_start(
                            uout[:, R * n : R * (n + 1)], u_sb[:]
                        )
                if rep_marker and mode != "nopv":
                    mark = mrkp.tile([1, 4], f32, tag="mark")
                    nc.vector.memset(mark[:], float(rep_i))
                    nc.sync.dma_start(uout[0:1, N * R : N * R + 4], mark[:])
    _split_sync_waits(nc)
    return nc


def _host_prep(x, A, gc_weight, bn_gamma, bn_beta, bn_mean, bn_var):
    x = np.asarray(x, np.float32)
    A = np.asarray(A, np.float32)
    W = np.asarray(gc_weight, np.float32)
    scale = np.asarray(bn_gamma, np.float32) / np.sqrt(
        np.asarray(bn_var, np.float32) + BN_EPS
    )
    d_half = 0.5 * np.eye(N, dtype=np.float32)
    a0 = np.ones((N, N), np.float32) - np.eye(N, dtype=np.float32)
    adj = d_half @ (a0 + A) @ d_half
    wk = 0.5 * (adj[0] + adj[1])                      # [N]
    cn = (wk * scale).astype(np.float32)              # [N]
    offset = float(
        np.sum(wk * (np.asarray(bn_beta, np.float32)
                     - np.asarray(bn_mean, np.float32) * scale))
    )
    bias_vec = (offset * W.sum(axis=0)).astype(np.float32)  # [D]

    bf = ml_dtypes.bfloat16
    xt = x.transpose(1, 2, 0)                         # [N, D, B] (view)
    # keys (x*C1 | C2 row) compacted per strip: strip r gets chunks
    # ck % N_ROWREP == r; queries (x | ones) full-B, slab cut in _in_maps
    NR = N_ROWREP
    xk = np.empty((N, DP, B), np.float32)
    xq_full = np.empty((N, DP, B), bf)
    for n in range(N):
        xk[n, :D, :] = xt[n] * C1
        xq_full[n, :D, :] = xt[n].astype(bf)
    xk[:, D, :] = np.float32(C2)
    xq_full[:, D, :] = np.float32(1.0)
    # [N, DP, KC, 128] -> strip-compacted [N, NR, DP, B//NR]
    xk = (
        xk.reshape(N, DP, KC, 128)
        .reshape(N, DP, KC // NR, NR, 128)
        .transpose(0, 3, 1, 2, 4)
        .reshape(N, NR, DP, B // NR)
        .astype(bf)
    )
    xh = np.empty((N, B, DP), np.float32)
    for n in range(N):
        xh[n, :, :D] = (x[:, n, :] @ W) * cn[n]
        xh[n, :, D] = 1.0
    # partition-major: [128, N*KC*DP] so each partition's DMA run is contiguous
    xh = np.ascontiguousarray(
        xh.reshape(N, KC, 128, DP).transpose(2, 0, 1, 3).reshape(128, N * KC * DP)
    ).astype(bf)
    return xk, xq_full, xh, bias_vec


def _in_maps(xk, xq_full, xh):
    NR = N_ROWREP
    maps = []
    for c in range(NCORES):
        q = xq_full[:, :, c * R : (c + 1) * R]          # [N, DP, R]
        qr = np.broadcast_to(q[:, None], (N, NR, DP, R))
        xkq = np.concatenate([xk, qr], axis=3)           # [N, NR, DP, KPS+R]
        maps.append({"xkq": np.ascontiguousarray(xkq), "xh": xh})
    return maps


def _finish(uouts, bias_vec):
    """Host gather: sum PV column-group partials (partition offsets 32g),
    divide by the folded denominator row, transpose, sum nodes, concat
    core slabs, add the BN/adjacency bias."""
    out = np.empty((B, D), np.float32)
    for c in range(NCORES):
        u = uouts[c]                                   # [128, N*R(+4)]
        acc = np.zeros((R, D), np.float32)
        for n in range(N):
            un = u[:, R * n : R * (n + 1)]             # [128, 512]
            tot = np.zeros((DP, R), np.float32)
            for g in range(N_COLGRP):
                tot += un[32 * g : 32 * g + DP, :]
            if not U_MEMSET:
                tot[0, 0] -= 1.0                       # dummy-MM clear token
            acc += (tot[:D] / tot[D]).T
        out[c * R : (c + 1) * R] = acc
    return out + bias_vec[None, :]


def kernel(**inputs) -> np.ndarray:
    assert inputs["x"].shape == (B, N, D)
    xk, xq_full, xh, bias_vec = _host_prep(**inputs)
    nc = build_nc(rep=1)
    res = run_bass_kernel_spmd(nc, _in_maps(xk, xq_full, xh), list(range(NCORES)))
    return _finish(
        [res.results[c]["uout"] for c in range(NCORES)], bias_vec
    ).astype(np.float32)
